# revision 9
# baseline (speedup 1.0000x reference)
"""CirLinear Trainium2 kernel: y = x @ build_weight(W, alphas, gumbels)^T + bias.

Strategy (8 NeuronCores, no collectives), 2x4 grid:
 - core c = tshard*4 + oshard: tokens [8192*tshard, +8192), out rows
   [512*oshard, +512)
 - circulant weight build done locally per core (512 rows, two 256-row
   chunks pipelined with the matmul)
 - x is passed host-transposed (xT slice [2048, 8192] f32) so the
   contraction dim lands on SBUF partitions with contiguous DMA; the
   f32->bf16 cast happens inside the load DMA (SWDGE)
 - bf16 matmul (lhsT = wT slice [128i,128o], rhs = xT tile [128i,512t])
   with fp32 PSUM accumulation over 16 K-chunks, bias added on the
   scalar engine, fp32 output out^T [512, 8192]
 - host assembles the 2x4 grid and transposes back
"""
import sys

sys.path.insert(0, '/opt/trn_rl_repo')

import numpy as np

import concourse.bass as bass
from concourse import bacc
import concourse.mybir as mybir
from concourse.tile import TileContext
from concourse.bass_utils import run_bass_kernel_spmd

N_CORES = 8
T_SHARDS, O_SHARDS = 2, 4
BATCH, TOKENS, IN_F, OUT_F = 16, 1024, 2048, 2048
TOK_TOTAL = BATCH * TOKENS            # 16384
TOK = TOK_TOTAL // T_SHARDS           # 8192 tokens per core
ROWS = OUT_F // O_SHARDS              # 512 out-features per core
N_CH = ROWS // 256                    # 2 build chunks of 256 rows
SCALES = [2, 4, 8, 16, 32, 64]
N_IC = IN_F // 128                    # 16 contraction chunks
N_TG = TOK // 512                     # 16 token groups of 512
N_OS = ROWS // 128                    # 4 output-row subtiles

bf16 = mybir.dt.bfloat16
f32 = mybir.dt.float32

_CACHE = {}


def _build_nc():
    nc = bacc.Bacc("TRN2", target_bir_lowering=False, debug=False, num_devices=N_CORES)
    xT = nc.dram_tensor("xT", [IN_F, TOK], f32, kind="ExternalInput")
    ws = nc.dram_tensor("ws", [ROWS, IN_F], f32, kind="ExternalInput")
    bias_s = nc.dram_tensor("bias_s", [1, ROWS], f32, kind="ExternalInput")
    alphas = nc.dram_tensor("alphas", [1, 7], f32, kind="ExternalInput")
    gumbels = nc.dram_tensor("gumbels", [1, 7], f32, kind="ExternalInput")
    out = nc.dram_tensor("out", [ROWS, TOK], f32, kind="ExternalOutput")

    w_loc = nc.dram_tensor("w_loc", [ROWS, IN_F], bf16)

    with TileContext(nc) as tc:
        # ---------- softmax(alphas + gumbels) broadcast to 128 partitions ----------
        asb = nc.alloc_sbuf_tensor("asb", [128, 7], f32).ap()
        gsb = nc.alloc_sbuf_tensor("gsb", [128, 7], f32).ap()
        a_bc = nc.alloc_sbuf_tensor("a_bc", [128, 7], f32).ap()
        ssum = nc.alloc_sbuf_tensor("ssum", [128, 1], f32).ap()
        nc.gpsimd.dma_start(out=asb, in_=bass.AP(tensor=alphas, offset=0, ap=[[0, 128], [1, 7]]))
        nc.gpsimd.dma_start(out=gsb, in_=bass.AP(tensor=gumbels, offset=0, ap=[[0, 128], [1, 7]]))
        nc.vector.tensor_tensor(out=asb, in0=asb, in1=gsb, op=mybir.AluOpType.add)
        nc.scalar.activation(out=asb, in_=asb, func=mybir.ActivationFunctionType.Exp)
        nc.vector.tensor_reduce(out=ssum, in_=asb, axis=mybir.AxisListType.X, op=mybir.AluOpType.add)
        nc.vector.reciprocal(out=ssum, in_=ssum)
        nc.vector.tensor_scalar_mul(a_bc, asb, ssum)

        # ---------- bias: [1, 512] -> [128 part, 4] (per-osub per-partition) ----------
        bias_sb = nc.alloc_sbuf_tensor("bias_sb", [128, N_OS], f32).ap()
        with nc.allow_non_contiguous_dma(reason="512-element one-time bias transpose"):
            nc.gpsimd.dma_start(out=bias_sb, in_=bass.AP(tensor=bias_s, offset=0, ap=[[1, 128], [128, N_OS]]))

        # ---------- circulant weight build: 2 chunks of 256 rows ----------
        # chunk partition = (q64, p64) : 4*32 = 128 ; free = (r64, s64) : 64*64
        wb = nc.alloc_sbuf_tensor("wb", [128, 4096], bf16).ap()
        acc = nc.alloc_sbuf_tensor("acc", [128, 4096], f32).ap()
        wbpad = nc.alloc_sbuf_tensor("wbpad", [128, 8192], bf16).ap()
        d_raw = nc.alloc_sbuf_tensor("d_raw", [128, 2048], f32).ap()
        dpad = nc.alloc_sbuf_tensor("dpad", [128, 4096], f32).ap()
        ws_4d = ws.ap().rearrange("(q r) (p s) -> q p r s", r=64, s=64)
        wloc_4d = w_loc.ap().rearrange("(q r) (p s) -> q p r s", r=64, s=64)

        wT = [nc.alloc_sbuf_tensor(f"wT{ic}", [128, ROWS], bf16).ap() for ic in range(N_IC)]

        def sb(t, off, dims):
            return bass.AP(tensor=t.tensor, offset=off, ap=[list(t.ap[0])] + dims)

        for ch in range(N_CH):
            for q in range(4):
                nc.gpsimd.dma_start(out=wb[q * 32:(q + 1) * 32, :], in_=ws_4d[ch * 4 + q])
            nc.vector.tensor_scalar_mul(acc, wb, a_bc[:, 0:1])
            for idx, b in enumerate(SCALES, start=1):
                nv = 64 // b
                src = sb(wb, 0, [[64, 64], [b, nv], [1, b]])
                for half in range(2):
                    dst = sb(wbpad, half * b, [[128, 64], [2 * b, nv], [1, b]])
                    nc.vector.tensor_copy(out=dst, in_=src)
                for u in range(nv):
                    rin = sb(wbpad, u * b * 128, [[2 * b, nv], [1, b], [129, b]])
                    rout = sb(d_raw, u * 64, [[b, nv], [1, b]])
                    nc.vector.tensor_reduce(out=rout, in_=rin, axis=mybir.AxisListType.X,
                                            op=mybir.AluOpType.add)
                dsrc = sb(d_raw, 0, [[64, nv], [b, nv], [1, b]])
                for half in range(2):
                    ddst = sb(dpad, half * b, [[128, nv], [2 * b, nv], [1, b]])
                    nc.vector.tensor_scalar(out=ddst, in0=dsrc, scalar1=a_bc[:, idx:idx + 1],
                                            scalar2=1.0 / b, op0=mybir.AluOpType.mult,
                                            op1=mybir.AluOpType.mult)
                for u in range(nv):
                    aout = sb(acc, u * b * 64, [[b, nv], [64, b], [1, b]])
                    din = sb(dpad, u * 128 + b, [[2 * b, nv], [-1, b], [1, b]])
                    nc.vector.tensor_tensor(out=aout, in0=aout, in1=din, op=mybir.AluOpType.add)
            # scatter chunk -> w_loc rows [256ch, 256ch+256) (bf16, SWDGE cast)
            for q in range(4):
                nc.gpsimd.dma_start(out=wloc_4d[ch * 4 + q], in_=acc[q * 32:(q + 1) * 32, :])
            # transposed reload of this chunk's columns into wT
            for ic in range(N_IC):
                nc.sync.dma_start(out=wT[ic][:, ch * 256:(ch + 1) * 256],
                                  in_=w_loc.ap()[ch * 256:(ch + 1) * 256, ic * 128:(ic + 1) * 128],
                                  transpose=True)

        # ---------- main matmul over 16 token groups ----------
        with (
            tc.tile_pool(name="xt", bufs=3) as xt_pool,
            tc.tile_pool(name="psum", bufs=2, space="PSUM") as psum_pool,
            tc.tile_pool(name="osb", bufs=8) as osb_pool,
        ):
            for tg in range(N_TG):
                # one SWDGE cast-DMA loads all 16 ic-chunks for this token group
                xt = xt_pool.tile([128, N_IC * 512], bf16, name="xt")
                nc.gpsimd.dma_start(
                    out=xt[:],
                    in_=bass.AP(tensor=xT, offset=tg * 512,
                                ap=[[TOK, 128], [128 * TOK, N_IC], [1, 512]]))
                psums = [psum_pool.tile([128, 512], f32, name=f"ps{o}", tag=f"ps{o}")
                         for o in range(N_OS)]
                for ic in range(N_IC):
                    rhs = xt[:, ic * 512:(ic + 1) * 512]
                    for o in range(N_OS):
                        nc.tensor.matmul(psums[o][:], wT[ic][:, o * 128:(o + 1) * 128], rhs,
                                         start=(ic == 0), stop=(ic == N_IC - 1))
                for o in range(N_OS):
                    ot = osb_pool.tile([128, 512], f32, name="ot")
                    nc.scalar.activation(out=ot[:], in_=psums[o][:],
                                         func=mybir.ActivationFunctionType.Identity,
                                         bias=bias_sb[:, o:o + 1], scale=1.0)
                    nc.sync.dma_start(out=out.ap()[o * 128:(o + 1) * 128, tg * 512:(tg + 1) * 512],
                                      in_=ot[:])

    nc.compile()
    return nc


def make_in_maps(x, weight, bias, alphas, gumbels):
    x2 = np.asarray(x, np.float32).reshape(TOK_TOTAL, IN_F)
    xTh = np.ascontiguousarray(x2.T)           # [2048, 16384]
    xslices = [np.ascontiguousarray(xTh[:, t * TOK:(t + 1) * TOK]) for t in range(T_SHARDS)]
    weight = np.asarray(weight, np.float32)
    bias = np.asarray(bias, np.float32)
    wslices = [np.ascontiguousarray(weight[o * ROWS:(o + 1) * ROWS]) for o in range(O_SHARDS)]
    bslices = [np.ascontiguousarray(bias[o * ROWS:(o + 1) * ROWS]).reshape(1, ROWS)
               for o in range(O_SHARDS)]
    al = np.asarray(alphas, np.float32).reshape(1, 7)
    gu = np.asarray(gumbels, np.float32).reshape(1, 7)
    in_maps = []
    for c in range(N_CORES):
        t, o = divmod(c, O_SHARDS)
        in_maps.append({"xT": xslices[t], "ws": wslices[o], "bias_s": bslices[o],
                        "alphas": al, "gumbels": gu})
    return in_maps


def kernel(x, weight, bias, alphas, gumbels):
    if "nc" not in _CACHE:
        _CACHE["nc"] = _build_nc()
    nc = _CACHE["nc"]
    in_maps = make_in_maps(x, weight, bias, alphas, gumbels)
    res = run_bass_kernel_spmd(nc, in_maps, core_ids=list(range(N_CORES)))
    # assemble: rows = o-shards, cols = t-shards
    row_blocks = []
    for o in range(O_SHARDS):
        row_blocks.append(np.concatenate(
            [res.results[t * O_SHARDS + o]["out"] for t in range(T_SHARDS)], axis=1))
    full_t = np.concatenate(row_blocks, axis=0)              # [2048, 16384]
    return np.ascontiguousarray(full_t.T).reshape(BATCH, TOKENS, OUT_F)


# revision 10
# speedup vs baseline: 1.0891x; 1.0891x over previous
"""CirLinear Trainium2 kernel: y = x @ build_weight(W, alphas, gumbels)^T + bias.

Strategy (8 NeuronCores, no collectives), 2x4 grid:
 - core c = tshard*4 + oshard: tokens [8192*tshard, +8192), out rows
   [512*oshard, +512)
 - circulant weight build done locally per core (512 rows, two 256-row
   chunks pipelined with the matmul)
 - x is passed host-transposed (xT slice [2048, 8192] f32) so the
   contraction dim lands on SBUF partitions with contiguous DMA; the
   f32->bf16 cast happens inside the load DMA (SWDGE)
 - bf16 matmul (lhsT = wT slice [128i,128o], rhs = xT tile [128i,512t])
   with fp32 PSUM accumulation over 16 K-chunks, bias added on the
   scalar engine, fp32 output out^T [512, 8192]
 - host assembles the 2x4 grid and transposes back
"""
import sys

sys.path.insert(0, '/opt/trn_rl_repo')

import numpy as np

import concourse.bass as bass
from concourse import bacc
import concourse.mybir as mybir
from concourse.tile import TileContext
from concourse.bass_utils import run_bass_kernel_spmd

N_CORES = 8
T_SHARDS, O_SHARDS = 2, 4
BATCH, TOKENS, IN_F, OUT_F = 16, 1024, 2048, 2048
TOK_TOTAL = BATCH * TOKENS            # 16384
TOK = TOK_TOTAL // T_SHARDS           # 8192 tokens per core
ROWS = OUT_F // O_SHARDS              # 512 out-features per core
N_CH = ROWS // 256                    # 2 build chunks of 256 rows
SCALES = [2, 4, 8, 16, 32, 64]
N_IC = IN_F // 128                    # 16 contraction chunks
N_TG = TOK // 512                     # 16 token groups of 512
N_OS = ROWS // 128                    # 4 output-row subtiles

bf16 = mybir.dt.bfloat16
f32 = mybir.dt.float32

_CACHE = {}


def _build_nc():
    nc = bacc.Bacc("TRN2", target_bir_lowering=False, debug=False, num_devices=N_CORES)
    xT = nc.dram_tensor("xT", [IN_F, TOK], f32, kind="ExternalInput")
    ws = nc.dram_tensor("ws", [ROWS, IN_F], f32, kind="ExternalInput")
    bias_s = nc.dram_tensor("bias_s", [1, ROWS], f32, kind="ExternalInput")
    alphas = nc.dram_tensor("alphas", [1, 7], f32, kind="ExternalInput")
    gumbels = nc.dram_tensor("gumbels", [1, 7], f32, kind="ExternalInput")
    out = nc.dram_tensor("out", [ROWS, TOK], f32, kind="ExternalOutput")

    w_loc = nc.dram_tensor("w_loc", [ROWS, IN_F], bf16)

    with TileContext(nc) as tc:
        # ---------- softmax(alphas + gumbels) broadcast to 128 partitions ----------
        asb = nc.alloc_sbuf_tensor("asb", [128, 7], f32).ap()
        gsb = nc.alloc_sbuf_tensor("gsb", [128, 7], f32).ap()
        a_bc = nc.alloc_sbuf_tensor("a_bc", [128, 7], f32).ap()
        ssum = nc.alloc_sbuf_tensor("ssum", [128, 1], f32).ap()
        nc.gpsimd.dma_start(out=asb, in_=bass.AP(tensor=alphas, offset=0, ap=[[0, 128], [1, 7]]))
        nc.gpsimd.dma_start(out=gsb, in_=bass.AP(tensor=gumbels, offset=0, ap=[[0, 128], [1, 7]]))
        nc.vector.tensor_tensor(out=asb, in0=asb, in1=gsb, op=mybir.AluOpType.add)
        nc.scalar.activation(out=asb, in_=asb, func=mybir.ActivationFunctionType.Exp)
        nc.vector.tensor_reduce(out=ssum, in_=asb, axis=mybir.AxisListType.X, op=mybir.AluOpType.add)
        nc.vector.reciprocal(out=ssum, in_=ssum)
        nc.vector.tensor_scalar_mul(a_bc, asb, ssum)

        # ---------- bias: [1, 512] -> [128 part, 4] (per-osub per-partition) ----------
        bias_sb = nc.alloc_sbuf_tensor("bias_sb", [128, N_OS], f32).ap()
        with nc.allow_non_contiguous_dma(reason="512-element one-time bias transpose"):
            nc.gpsimd.dma_start(out=bias_sb, in_=bass.AP(tensor=bias_s, offset=0, ap=[[1, 128], [128, N_OS]]))

        # ---------- circulant weight build: 2 chunks of 256 rows ----------
        # chunk partition = (q64, p64) : 4*32 = 128 ; free = (r64, s64) : 64*64
        wb = nc.alloc_sbuf_tensor("wb", [128, 4096], bf16).ap()
        acc = nc.alloc_sbuf_tensor("acc", [128, 4096], f32).ap()
        wbpad = nc.alloc_sbuf_tensor("wbpad", [128, 8192], bf16).ap()
        d_raw = nc.alloc_sbuf_tensor("d_raw", [128, 2048], f32).ap()
        dpad = nc.alloc_sbuf_tensor("dpad", [128, 4096], f32).ap()
        ws_4d = ws.ap().rearrange("(q r) (p s) -> q p r s", r=64, s=64)
        wloc_4d = w_loc.ap().rearrange("(q r) (p s) -> q p r s", r=64, s=64)

        wT = [nc.alloc_sbuf_tensor(f"wT{ic}", [128, ROWS], bf16).ap() for ic in range(N_IC)]

        def sb(t, off, dims):
            return bass.AP(tensor=t.tensor, offset=off, ap=[list(t.ap[0])] + dims)

        for ch in range(N_CH):
            for q in range(4):
                nc.gpsimd.dma_start(out=wb[q * 32:(q + 1) * 32, :], in_=ws_4d[ch * 4 + q])
            nc.vector.tensor_scalar_mul(acc, wb, a_bc[:, 0:1])
            for idx, b in enumerate(SCALES, start=1):
                nv = 64 // b
                src = sb(wb, 0, [[64, 64], [b, nv], [1, b]])
                for half in range(2):
                    dst = sb(wbpad, half * b, [[128, 64], [2 * b, nv], [1, b]])
                    nc.scalar.copy(out=dst, in_=src)
                for u in range(nv):
                    rin = sb(wbpad, u * b * 128, [[2 * b, nv], [1, b], [129, b]])
                    rout = sb(d_raw, u * 64, [[b, nv], [1, b]])
                    nc.vector.tensor_reduce(out=rout, in_=rin, axis=mybir.AxisListType.X,
                                            op=mybir.AluOpType.add)
                dsrc = sb(d_raw, 0, [[64, nv], [b, nv], [1, b]])
                for half in range(2):
                    ddst = sb(dpad, half * b, [[128, nv], [2 * b, nv], [1, b]])
                    nc.vector.tensor_scalar(out=ddst, in0=dsrc, scalar1=a_bc[:, idx:idx + 1],
                                            scalar2=1.0 / b, op0=mybir.AluOpType.mult,
                                            op1=mybir.AluOpType.mult)
                for u in range(nv):
                    aout = sb(acc, u * b * 64, [[b, nv], [64, b], [1, b]])
                    din = sb(dpad, u * 128 + b, [[2 * b, nv], [-1, b], [1, b]])
                    nc.vector.tensor_tensor(out=aout, in0=aout, in1=din, op=mybir.AluOpType.add)
            # scatter chunk -> w_loc rows [256ch, 256ch+256) (bf16, SWDGE cast)
            for q in range(4):
                nc.gpsimd.dma_start(out=wloc_4d[ch * 4 + q], in_=acc[q * 32:(q + 1) * 32, :])
            # transposed reload of this chunk's columns into wT
            for ic in range(N_IC):
                nc.sync.dma_start(out=wT[ic][:, ch * 256:(ch + 1) * 256],
                                  in_=w_loc.ap()[ch * 256:(ch + 1) * 256, ic * 128:(ic + 1) * 128],
                                  transpose=True)

        # ---------- main matmul over 16 token groups ----------
        with (
            tc.tile_pool(name="xt", bufs=3) as xt_pool,
            tc.tile_pool(name="psum", bufs=2, space="PSUM") as psum_pool,
            tc.tile_pool(name="osb", bufs=8) as osb_pool,
        ):
            for tg in range(N_TG):
                # one SWDGE cast-DMA loads all 16 ic-chunks for this token group
                xt = xt_pool.tile([128, N_IC * 512], bf16, name="xt")
                nc.gpsimd.dma_start(
                    out=xt[:],
                    in_=bass.AP(tensor=xT, offset=tg * 512,
                                ap=[[TOK, 128], [128 * TOK, N_IC], [1, 512]]))
                psums = [psum_pool.tile([128, 512], f32, name=f"ps{o}", tag=f"ps{o}")
                         for o in range(N_OS)]
                for ic in range(N_IC):
                    rhs = xt[:, ic * 512:(ic + 1) * 512]
                    for o in range(N_OS):
                        nc.tensor.matmul(psums[o][:], wT[ic][:, o * 128:(o + 1) * 128], rhs,
                                         start=(ic == 0), stop=(ic == N_IC - 1))
                for o in range(N_OS):
                    ot = osb_pool.tile([128, 512], f32, name="ot")
                    nc.vector.tensor_scalar_add(ot[:], psums[o][:], bias_sb[:, o:o + 1])
                    nc.sync.dma_start(out=out.ap()[o * 128:(o + 1) * 128, tg * 512:(tg + 1) * 512],
                                      in_=ot[:])

    nc.compile()
    return nc


def make_in_maps(x, weight, bias, alphas, gumbels):
    x2 = np.asarray(x, np.float32).reshape(TOK_TOTAL, IN_F)
    xTh = np.ascontiguousarray(x2.T)           # [2048, 16384]
    xslices = [np.ascontiguousarray(xTh[:, t * TOK:(t + 1) * TOK]) for t in range(T_SHARDS)]
    weight = np.asarray(weight, np.float32)
    bias = np.asarray(bias, np.float32)
    wslices = [np.ascontiguousarray(weight[o * ROWS:(o + 1) * ROWS]) for o in range(O_SHARDS)]
    bslices = [np.ascontiguousarray(bias[o * ROWS:(o + 1) * ROWS]).reshape(1, ROWS)
               for o in range(O_SHARDS)]
    al = np.asarray(alphas, np.float32).reshape(1, 7)
    gu = np.asarray(gumbels, np.float32).reshape(1, 7)
    in_maps = []
    for c in range(N_CORES):
        t, o = divmod(c, O_SHARDS)
        in_maps.append({"xT": xslices[t], "ws": wslices[o], "bias_s": bslices[o],
                        "alphas": al, "gumbels": gu})
    return in_maps


def kernel(x, weight, bias, alphas, gumbels):
    if "nc" not in _CACHE:
        _CACHE["nc"] = _build_nc()
    nc = _CACHE["nc"]
    in_maps = make_in_maps(x, weight, bias, alphas, gumbels)
    res = run_bass_kernel_spmd(nc, in_maps, core_ids=list(range(N_CORES)))
    # assemble: rows = o-shards, cols = t-shards
    row_blocks = []
    for o in range(O_SHARDS):
        row_blocks.append(np.concatenate(
            [res.results[t * O_SHARDS + o]["out"] for t in range(T_SHARDS)], axis=1))
    full_t = np.concatenate(row_blocks, axis=0)              # [2048, 16384]
    return np.ascontiguousarray(full_t.T).reshape(BATCH, TOKENS, OUT_F)


# revision 12
# speedup vs baseline: 1.6079x; 1.4763x over previous
"""CirLinear Trainium2 kernel: y = x @ build_weight(W, alphas, gumbels)^T + bias.

Strategy (8 NeuronCores, no collectives), 2x4 grid:
 - core c = tshard*4 + oshard: tokens [8192*tshard, +8192), out rows
   [512*oshard, +512)
 - circulant weight build done locally per core (512 rows, two 256-row
   chunks pipelined with the matmul)
 - x is passed host-transposed (xT slice [2048, 8192] f32) so the
   contraction dim lands on SBUF partitions with contiguous DMA; the
   f32->bf16 cast happens inside the load DMA (SWDGE)
 - bf16 matmul (lhsT = wT slice [128i,128o], rhs = xT tile [128i,512t])
   with fp32 PSUM accumulation over 16 K-chunks, bias added on the
   scalar engine, fp32 output out^T [512, 8192]
 - host assembles the 2x4 grid and transposes back
"""
import sys

sys.path.insert(0, '/opt/trn_rl_repo')

import numpy as np

import concourse.bass as bass
from concourse import bacc
import concourse.mybir as mybir
from concourse.tile import TileContext
from concourse.bass_utils import run_bass_kernel_spmd

N_CORES = 8
T_SHARDS, O_SHARDS = 2, 4
BATCH, TOKENS, IN_F, OUT_F = 16, 1024, 2048, 2048
TOK_TOTAL = BATCH * TOKENS            # 16384
TOK = TOK_TOTAL // T_SHARDS           # 8192 tokens per core
ROWS = OUT_F // O_SHARDS              # 512 out-features per core
N_CH = ROWS // 256                    # 2 build chunks of 256 rows
SCALES = [2, 4, 8, 16, 32, 64]
N_IC = IN_F // 128                    # 16 contraction chunks
N_TG = TOK // 512                     # 16 token groups of 512
N_OS = ROWS // 128                    # 4 output-row subtiles

bf16 = mybir.dt.bfloat16
f32 = mybir.dt.float32

_CACHE = {}


def _build_nc():
    nc = bacc.Bacc("TRN2", target_bir_lowering=False, debug=False, num_devices=N_CORES)
    xT = nc.dram_tensor("xT", [IN_F, TOK], bf16, kind="ExternalInput")
    ws = nc.dram_tensor("ws", [ROWS, IN_F], bf16, kind="ExternalInput")
    bias_s = nc.dram_tensor("bias_s", [1, ROWS], f32, kind="ExternalInput")
    alphas = nc.dram_tensor("alphas", [1, 7], f32, kind="ExternalInput")
    gumbels = nc.dram_tensor("gumbels", [1, 7], f32, kind="ExternalInput")
    out = nc.dram_tensor("out", [ROWS, TOK], f32, kind="ExternalOutput")

    w_loc = nc.dram_tensor("w_loc", [ROWS, IN_F], bf16)

    with TileContext(nc) as tc:
        # ---------- softmax(alphas + gumbels) broadcast to 128 partitions ----------
        asb = nc.alloc_sbuf_tensor("asb", [128, 7], f32).ap()
        gsb = nc.alloc_sbuf_tensor("gsb", [128, 7], f32).ap()
        a_bc = nc.alloc_sbuf_tensor("a_bc", [128, 7], f32).ap()
        ssum = nc.alloc_sbuf_tensor("ssum", [128, 1], f32).ap()
        nc.gpsimd.dma_start(out=asb, in_=bass.AP(tensor=alphas, offset=0, ap=[[0, 128], [1, 7]]))
        nc.gpsimd.dma_start(out=gsb, in_=bass.AP(tensor=gumbels, offset=0, ap=[[0, 128], [1, 7]]))
        nc.vector.tensor_tensor(out=asb, in0=asb, in1=gsb, op=mybir.AluOpType.add)
        nc.scalar.activation(out=asb, in_=asb, func=mybir.ActivationFunctionType.Exp)
        nc.vector.tensor_reduce(out=ssum, in_=asb, axis=mybir.AxisListType.X, op=mybir.AluOpType.add)
        nc.vector.reciprocal(out=ssum, in_=ssum)
        nc.vector.tensor_scalar_mul(a_bc, asb, ssum)

        # ---------- bias: [1, 512] -> [128 part, 4] (per-osub per-partition) ----------
        bias_sb = nc.alloc_sbuf_tensor("bias_sb", [128, N_OS], f32).ap()
        with nc.allow_non_contiguous_dma(reason="512-element one-time bias transpose"):
            nc.gpsimd.dma_start(out=bias_sb, in_=bass.AP(tensor=bias_s, offset=0, ap=[[1, 128], [128, N_OS]]))

        # ---------- circulant weight build: 2 chunks of 256 rows ----------
        # chunk partition = (q64, p64) : 4*32 = 128 ; free = (r64, s64) : 64*64
        wb = nc.alloc_sbuf_tensor("wb", [128, 4096], bf16).ap()
        acc = nc.alloc_sbuf_tensor("acc", [128, 4096], f32).ap()
        wbpad = nc.alloc_sbuf_tensor("wbpad", [128, 8192], bf16).ap()
        d_raw = nc.alloc_sbuf_tensor("d_raw", [128, 2048], f32).ap()
        dpad = nc.alloc_sbuf_tensor("dpad", [128, 4096], f32).ap()
        ws_4d = ws.ap().rearrange("(q r) (p s) -> q p r s", r=64, s=64)
        wloc_4d = w_loc.ap().rearrange("(q r) (p s) -> q p r s", r=64, s=64)

        wT = [nc.alloc_sbuf_tensor(f"wT{ic}", [128, ROWS], bf16).ap() for ic in range(N_IC)]

        def sb(t, off, dims):
            return bass.AP(tensor=t.tensor, offset=off, ap=[list(t.ap[0])] + dims)

        for ch in range(N_CH):
            for q in range(4):
                nc.sync.dma_start(out=wb[q * 32:(q + 1) * 32, :], in_=ws_4d[ch * 4 + q])
            nc.vector.tensor_scalar_mul(acc, wb, a_bc[:, 0:1])
            for idx, b in enumerate(SCALES, start=1):
                nv = 64 // b
                src = sb(wb, 0, [[64, 64], [b, nv], [1, b]])
                for half in range(2):
                    dst = sb(wbpad, half * b, [[128, 64], [2 * b, nv], [1, b]])
                    nc.scalar.copy(out=dst, in_=src)
                for u in range(nv):
                    rin = sb(wbpad, u * b * 128, [[2 * b, nv], [1, b], [129, b]])
                    rout = sb(d_raw, u * 64, [[b, nv], [1, b]])
                    nc.vector.tensor_reduce(out=rout, in_=rin, axis=mybir.AxisListType.X,
                                            op=mybir.AluOpType.add)
                dsrc = sb(d_raw, 0, [[64, nv], [b, nv], [1, b]])
                for half in range(2):
                    ddst = sb(dpad, half * b, [[128, nv], [2 * b, nv], [1, b]])
                    nc.vector.tensor_scalar(out=ddst, in0=dsrc, scalar1=a_bc[:, idx:idx + 1],
                                            scalar2=1.0 / b, op0=mybir.AluOpType.mult,
                                            op1=mybir.AluOpType.mult)
                for u in range(nv):
                    aout = sb(acc, u * b * 64, [[b, nv], [64, b], [1, b]])
                    din = sb(dpad, u * 128 + b, [[2 * b, nv], [-1, b], [1, b]])
                    nc.vector.tensor_tensor(out=aout, in0=aout, in1=din, op=mybir.AluOpType.add)
            # scatter chunk -> w_loc rows [256ch, 256ch+256) (bf16, SWDGE cast)
            for q in range(4):
                nc.gpsimd.dma_start(out=wloc_4d[ch * 4 + q], in_=acc[q * 32:(q + 1) * 32, :])
            # transposed reload of this chunk's columns into wT
            for ic in range(N_IC):
                nc.sync.dma_start(out=wT[ic][:, ch * 256:(ch + 1) * 256],
                                  in_=w_loc.ap()[ch * 256:(ch + 1) * 256, ic * 128:(ic + 1) * 128],
                                  transpose=True)

        # ---------- main matmul over 16 token groups ----------
        with (
            tc.tile_pool(name="xt", bufs=3) as xt_pool,
            tc.tile_pool(name="psum", bufs=2, space="PSUM") as psum_pool,
            tc.tile_pool(name="osb", bufs=8) as osb_pool,
        ):
            for tg in range(N_TG):
                # one SWDGE cast-DMA loads all 16 ic-chunks for this token group
                xt = xt_pool.tile([128, N_IC * 512], bf16, name="xt")
                nc.sync.dma_start(
                    out=xt[:],
                    in_=bass.AP(tensor=xT, offset=tg * 512,
                                ap=[[TOK, 128], [128 * TOK, N_IC], [1, 512]]))
                psums = [psum_pool.tile([128, 512], f32, name=f"ps{o}", tag=f"ps{o}")
                         for o in range(N_OS)]
                for ic in range(N_IC):
                    rhs = xt[:, ic * 512:(ic + 1) * 512]
                    for o in range(N_OS):
                        nc.tensor.matmul(psums[o][:], wT[ic][:, o * 128:(o + 1) * 128], rhs,
                                         start=(ic == 0), stop=(ic == N_IC - 1))
                for o in range(N_OS):
                    ot = osb_pool.tile([128, 512], f32, name="ot")
                    nc.scalar.activation(out=ot[:], in_=psums[o][:],
                                         func=mybir.ActivationFunctionType.Identity,
                                         bias=bias_sb[:, o:o + 1], scale=1.0)
                    nc.sync.dma_start(out=out.ap()[o * 128:(o + 1) * 128, tg * 512:(tg + 1) * 512],
                                      in_=ot[:])

    nc.compile()
    return nc


def make_in_maps(x, weight, bias, alphas, gumbels):
    import ml_dtypes
    x2 = np.asarray(x, np.float32).reshape(TOK_TOTAL, IN_F)
    xTh = np.ascontiguousarray(x2.T).astype(ml_dtypes.bfloat16)   # [2048, 16384]
    xslices = [np.ascontiguousarray(xTh[:, t * TOK:(t + 1) * TOK]) for t in range(T_SHARDS)]
    weight = np.asarray(weight, np.float32)
    bias = np.asarray(bias, np.float32)
    wslices = [np.ascontiguousarray(weight[o * ROWS:(o + 1) * ROWS]).astype(ml_dtypes.bfloat16)
               for o in range(O_SHARDS)]
    bslices = [np.ascontiguousarray(bias[o * ROWS:(o + 1) * ROWS]).reshape(1, ROWS)
               for o in range(O_SHARDS)]
    al = np.asarray(alphas, np.float32).reshape(1, 7)
    gu = np.asarray(gumbels, np.float32).reshape(1, 7)
    in_maps = []
    for c in range(N_CORES):
        t, o = divmod(c, O_SHARDS)
        in_maps.append({"xT": xslices[t], "ws": wslices[o], "bias_s": bslices[o],
                        "alphas": al, "gumbels": gu})
    return in_maps


def kernel(x, weight, bias, alphas, gumbels):
    if "nc" not in _CACHE:
        _CACHE["nc"] = _build_nc()
    nc = _CACHE["nc"]
    in_maps = make_in_maps(x, weight, bias, alphas, gumbels)
    res = run_bass_kernel_spmd(nc, in_maps, core_ids=list(range(N_CORES)))
    # assemble: rows = o-shards, cols = t-shards
    row_blocks = []
    for o in range(O_SHARDS):
        row_blocks.append(np.concatenate(
            [res.results[t * O_SHARDS + o]["out"] for t in range(T_SHARDS)], axis=1))
    full_t = np.concatenate(row_blocks, axis=0)              # [2048, 16384]
    return np.ascontiguousarray(full_t.T).reshape(BATCH, TOKENS, OUT_F)
